# revision 7
# baseline (speedup 1.0000x reference)
"""Trainium2 Bass kernel for nn_PennylaneCircuit: 18-qubit statevector circuit,
6 layers of per-wire RX/RY/RZ + nearest-neighbor CNOT chain, measuring <Z_0>.

Math: split wires 0-8 (rows) | 9-17 (cols); the state is a 512x512 complex
matrix S. Per layer, all row-space gates fuse into one 512x512 operator L_l
(host-precomputed from the angles), all col-space gates into M_l, and the only
cross-cut gate CNOT(8,9) makes the Schmidt rank double. So S = U @ V^T with
rank <= 32 factors; the device only ever multiplies 512x512 operators into
512x{2..32} factors and computes the final <Z_0> from the Gram matrix of V.

Device data layout: factors packed as f32 [real | imag] per 128-row tile;
complex matmul = two PSUM-accumulated passes (weights L_r, L_i transposed on
host) against packed rhs [Ur|Ui] and [-Ui|Ur].
"""

import numpy as np

NQ = 18
DEPTH = 6
A = 9            # row-group wires 0..8; col group 9..17
DIM = 1 << A     # 512
NK = DIM // 128  # 4 partition tiles per factor


# ---------------------------------------------------------------- host math

def _rx(t):
    c, s = np.cos(t / 2), np.sin(t / 2)
    return np.array([[c, -1j * s], [-1j * s, c]])


def _ry(t):
    c, s = np.cos(t / 2), np.sin(t / 2)
    return np.array([[c, -s], [s, c]])


def _rz(t):
    return np.array([[np.exp(-1j * t / 2), 0], [0, np.exp(1j * t / 2)]])


def _fused_1q(angles):
    # circuit order RX, RY, RZ  =>  matrix RZ @ RY @ RX
    return _rz(angles[2]) @ _ry(angles[1]) @ _rx(angles[0])


def _kron_all(mats):
    out = np.array([[1.0 + 0j]])
    for m in mats:
        out = np.kron(out, m)
    return out


def _cnot_chain_op(n, pairs):
    """Permutation matrix applying CNOT(w, w+1) for w in pairs, in order.

    Wire w of an n-wire group sits at bit (n-1-w) of the index."""
    dim = 1 << n
    perm = np.arange(dim)
    for w in pairs:
        cb, tb = n - 1 - w, n - 2 - w
        ctrl = (perm >> cb) & 1
        perm = np.where(ctrl == 1, perm ^ (1 << tb), perm)
    op = np.zeros((dim, dim), dtype=np.complex128)
    op[perm, np.arange(dim)] = 1
    return op


def _build_ops(params, basis):
    """L[0..5] row-space ops, M[0..4] col-space ops (M[5] never affects <Z_0>)."""
    params = np.asarray(params, np.float64)
    basis = np.asarray(basis, np.float64)
    Cleft = _cnot_chain_op(A, range(A - 1))
    Cright = _cnot_chain_op(NQ - A, range(NQ - A - 1))
    Ab = _kron_all([_fused_1q(basis[w]) for w in range(A)])
    Bb = _kron_all([_fused_1q(basis[A + w]) for w in range(NQ - A)])
    L, M = [], []
    for l in range(DEPTH):
        Rl = _kron_all([_fused_1q(params[l, w]) for w in range(A)])
        Rt = _kron_all([_fused_1q(params[l, A + w]) for w in range(NQ - A)])
        Al = Cleft @ Rl
        if l == 0:
            L.append(Al @ Ab)
            M.append(Rt @ Bb)
        else:
            L.append(Al)
            M.append(Rt @ Cright)
    return L, M


def build_host_data(params, basis):
    """Everything the device needs, all float32."""
    L, M = _build_ops(params, basis)
    u1 = L[0][:, 0]            # state after layer 1 = L1 e0 (x) M1 e0
    v1 = M[0][:, 0]
    # crossing X_1: U -> [mask_e*u1 | mask_o*u1],  V -> [v1 | P v1]
    idx = np.arange(DIM)
    me = (idx % 2 == 0).astype(np.float64)
    U2 = np.stack([u1 * me, u1 * (1 - me)], axis=1)
    V2 = np.stack([v1, v1[idx ^ 256]], axis=1)

    d = {
        "ua0": _pack_pair(U2.real, U2.imag),
        "ub0": _pack_pair(-U2.imag, U2.real),
        "va0": _pack_pair(V2.real, V2.imag),
        "vb0": _pack_pair(-V2.imag, V2.real),
    }

    d["wLr"] = np.stack([L[l].T.real for l in range(1, 6)]).astype(np.float32)
    d["wLi"] = np.stack([L[l].T.imag for l in range(1, 6)]).astype(np.float32)
    d["wMr"] = np.stack([M[l].T.real for l in range(1, 5)]).astype(np.float32)
    d["wMi"] = np.stack([M[l].T.imag for l in range(1, 5)]).astype(np.float32)

    masks = np.zeros((128, 8), np.float32)
    p = np.arange(128)
    masks[:, 0] = (p % 2 == 0)          # mE
    masks[:, 1] = (p % 2 == 1)          # mO
    masks[:, 2] = -masks[:, 0]          # -mE
    masks[:, 3] = -masks[:, 1]          # -mO
    masks[:, 4] = 1.0                   # ones
    d["masks"] = masks
    d["ident"] = np.eye(128, dtype=np.float32)
    return d


def _pack_pair(first, second):
    """two (512, r) real arrays -> (128, NK, 2r) f32 [first | second]."""
    r = first.shape[1]
    out = np.empty((128, NK, 2 * r), np.float32)
    for k in range(NK):
        out[:, k, :r] = first[k * 128:(k + 1) * 128]
        out[:, k, r:] = second[k * 128:(k + 1) * 128]
    return out


# ------------------------------------------------- numpy mirror of the device

def device_sim(d):
    """Mirrors the Bass op schedule exactly (packed tiles, two-pass cmms)."""
    wLr, wLi, wMr, wMi = d["wLr"], d["wLi"], d["wMr"], d["wMi"]
    masks = d["masks"]
    mE, mO = masks[:, 0:1], masks[:, 1:2]

    Ua, Ub = d["ua0"].copy(), d["ub0"].copy()
    Va, Vb = d["va0"].copy(), d["vb0"].copy()

    def cmm(wr, wi, Xa, Xb, r):
        """psum[m] = sum_k wr[k,m].T @ Xa[k] + wi[k,m].T @ Xb[k],  (128, 2r) each."""
        w = 2 * r
        ps = []
        for m in range(NK):
            acc = np.zeros((128, w), np.float32)
            for k in range(NK):
                lr = wr[k * 128:(k + 1) * 128, m * 128:(m + 1) * 128]
                li = wi[k * 128:(k + 1) * 128, m * 128:(m + 1) * 128]
                acc += lr.T @ Xa[:, k, :w] + li.T @ Xb[:, k, :w]
            ps.append(acc)
        return ps

    for li in range(5):
        r = 2 << li          # rank during this layer's cmm (2,4,8,16,32)
        w = 2 * r
        psU = cmm(wLr[li], wLi[li], Ua, Ub, r)
        if li < 4:
            psV = cmm(wMr[li], wMi[li], Va, Vb, r)
            Ua2 = np.zeros((128, NK, 4 * r), np.float32)
            Ub2 = np.zeros_like(Ua2)
            Va2 = np.zeros_like(Ua2)
            Vb2 = np.zeros_like(Ua2)
            for m in range(NK):
                pv = psU[m].reshape(128, 2, r)           # [Cr | Ci]
                oA = Ua2[:, m].reshape(128, 2, 2 * r)    # [Ur' | Ui']
                oA[:, :, 0:r] = pv * mE[:, None]
                oA[:, :, r:] = pv * mO[:, None]
                oB = Ub2[:, m].reshape(128, 2, 2 * r)    # [-Ui' | Ur']
                oB[:, 0, 0:r] = pv[:, 1] * -mE
                oB[:, 0, r:] = pv[:, 1] * -mO
                oB[:, 1, :] = oA[:, 0]
                # V: Va2[m] gets [Cr| ...] halves, Va2[m^2] the P-copy
                ps = psV[m].reshape(128, 2, r)
                Va2[:, m].reshape(128, 2, 2 * r)[:, :, 0:r] = ps
                Va2[:, m ^ 2].reshape(128, 2, 2 * r)[:, :, r:] = ps
                Vb2[:, m, 0:r] = -psV[m][:, r:]
                Vb2[:, m, 2 * r:3 * r] = psV[m][:, 0:r]
                Vb2[:, m ^ 2, r:2 * r] = -psV[m][:, r:]
                Vb2[:, m ^ 2, 3 * r:] = psV[m][:, 0:r]
            Ua, Ub, Va, Vb = Ua2, Ub2, Va2, Vb2
        else:
            Uf = np.zeros((128, NK, w), np.float32)
            for m in range(NK):
                Uf[:, m] = psU[m]

    # G accumulation: psG = [-Gi | Gr]  (32, 64)
    R = 32
    psG = np.zeros((R, 2 * R), np.float32)
    for k in range(NK):
        Vr, Vi = Va[:, k, :R], Va[:, k, R:]
        psG += Vr.T @ Vb[:, k] + Vi.T @ Va[:, k]
    Gy = psG                                  # [-Gi | Gr]
    Gx = np.concatenate([psG[:, R:], -psG[:, :R]], 1)   # [Gr | Gi]

    total = np.zeros((128, 1), np.float32)
    for m in range(NK):
        UrT = Uf[:, m, :R].T                  # (32, 128)
        UiT = Uf[:, m, R:].T
        psW = UrT.T @ Gx + UiT.T @ Gy         # [Wr | Wi] (128, 64)
        t = psW * Uf[:, m]                    # elementwise both halves
        nm = t.sum(axis=1, keepdims=True)
        total = total + nm if m < 2 else total - nm
    return np.float32(total.sum())


# ------------------------------------------------------------- bass program

_CACHE = {}


def _build_bass():
    import concourse.mybir as mybir
    import concourse.tile as tile
    from concourse import bacc

    f32 = mybir.dt.float32
    mult = mybir.AluOpType.mult
    X = mybir.AxisListType.X

    nc = bacc.Bacc("TRN2", target_bir_lowering=False)
    wLr_d = nc.dram_tensor("wLr", (5, 512, 512), f32, kind="ExternalInput")
    wLi_d = nc.dram_tensor("wLi", (5, 512, 512), f32, kind="ExternalInput")
    wMr_d = nc.dram_tensor("wMr", (4, 512, 512), f32, kind="ExternalInput")
    wMi_d = nc.dram_tensor("wMi", (4, 512, 512), f32, kind="ExternalInput")
    ua0_d = nc.dram_tensor("ua0", (128, NK, 4), f32, kind="ExternalInput")
    ub0_d = nc.dram_tensor("ub0", (128, NK, 4), f32, kind="ExternalInput")
    va0_d = nc.dram_tensor("va0", (128, NK, 4), f32, kind="ExternalInput")
    vb0_d = nc.dram_tensor("vb0", (128, NK, 4), f32, kind="ExternalInput")
    masks_d = nc.dram_tensor("masks", (128, 8), f32, kind="ExternalInput")
    ident_d = nc.dram_tensor("ident", (128, 128), f32, kind="ExternalInput")
    out_d = nc.dram_tensor("out", (1, 1), f32, kind="ExternalOutput")

    with tile.TileContext(nc) as tc:
        with (
            tc.tile_pool(name="consts", bufs=1) as cpool,
            tc.tile_pool(name="weights", bufs=2) as wpool,
            tc.tile_pool(name="state", bufs=2) as spool,
            tc.tile_pool(name="fin", bufs=4) as fpool,
            tc.tile_pool(name="psum", bufs=8, space="PSUM") as pspool,
        ):
            masks_t = cpool.tile([128, 8], f32)
            nc.sync.dma_start(masks_t[:], masks_d[:, :])
            ident_t = cpool.tile([128, 128], f32)
            nc.sync.dma_start(ident_t[:], ident_d[:, :])

            Ua = spool.tile([128, NK, 64], f32, tag="Ua")
            Ub = spool.tile([128, NK, 64], f32, tag="Ub")
            Va = spool.tile([128, NK, 64], f32, tag="Va")
            Vb = spool.tile([128, NK, 64], f32, tag="Vb")
            nc.sync.dma_start(Ua[:, :, 0:4], ua0_d[:, :, :])
            nc.sync.dma_start(Ub[:, :, 0:4], ub0_d[:, :, :])
            nc.sync.dma_start(Va[:, :, 0:4], va0_d[:, :, :])
            nc.sync.dma_start(Vb[:, :, 0:4], vb0_d[:, :, :])

            def load_w(dram, li, tag):
                t = wpool.tile([128, NK, 512], f32, tag=tag)
                nc.sync.dma_start(
                    t[:], dram[li, :, :].rearrange("(ko p) m -> p ko m", p=128)
                )
                return t

            def cmm(wr, wi, Xa, Xb, r, tag):
                w = 2 * r
                ps = []
                for m in range(NK):
                    pt = pspool.tile([128, 64], f32, tag="ps", name=f"{tag}{m}")
                    for k in range(NK):
                        nc.tensor.matmul(
                            pt[:, 0:w],
                            lhsT=wr[:, k, m * 128:(m + 1) * 128],
                            rhs=Xa[:, k, 0:w],
                            start=(k == 0),
                            stop=False,
                        )
                        nc.tensor.matmul(
                            pt[:, 0:w],
                            lhsT=wi[:, k, m * 128:(m + 1) * 128],
                            rhs=Xb[:, k, 0:w],
                            start=False,
                            stop=(k == NK - 1),
                        )
                    ps.append(pt)
                return ps

            mE = masks_t[:, 0:1, None]
            mO = masks_t[:, 1:2, None]
            mEn = masks_t[:, 2:3]
            mOn = masks_t[:, 3:4]

            Uf = None
            for li in range(5):
                r = 2 << li
                w = 2 * r
                wr = load_w(wLr_d, li, "wLr")
                wi = load_w(wLi_d, li, "wLi")
                psU = cmm(wr, wi, Ua, Ub, r, f"u{li}")
                if li < 4:
                    mr = load_w(wMr_d, li, "wMr")
                    mi = load_w(wMi_d, li, "wMi")
                    psV = cmm(mr, mi, Va, Vb, r, f"v{li}")
                    Ua2 = spool.tile([128, NK, 64], f32, tag="Ua")
                    Ub2 = spool.tile([128, NK, 64], f32, tag="Ub")
                    Va2 = spool.tile([128, NK, 64], f32, tag="Va")
                    Vb2 = spool.tile([128, NK, 64], f32, tag="Vb")
                    for m in range(NK):
                        pv = psU[m][:, 0:w].rearrange("p (h x) -> p h x", h=2)
                        oA = Ua2[:, m, 0:2 * w].rearrange("p (h x) -> p h x", h=2)
                        nc.vector.tensor_tensor(
                            out=oA[:, :, 0:r], in0=pv, in1=mE.to_broadcast((128, 2, r)),
                            op=mult,
                        )
                        nc.vector.tensor_tensor(
                            out=oA[:, :, r:2 * r], in0=pv,
                            in1=mO.to_broadcast((128, 2, r)), op=mult,
                        )
                        oB = Ub2[:, m, 0:2 * w].rearrange("p (h x) -> p h x", h=2)
                        nc.vector.tensor_tensor(
                            out=oB[:, 0, 0:r], in0=psU[m][:, r:w],
                            in1=mEn.to_broadcast((128, r)), op=mult,
                        )
                        nc.vector.tensor_tensor(
                            out=oB[:, 0, r:2 * r], in0=psU[m][:, r:w],
                            in1=mOn.to_broadcast((128, r)), op=mult,
                        )
                        nc.vector.tensor_copy(out=oB[:, 1, :], in_=oA[:, 0, :])
                        # V side: doubled copyback [V | PV]
                        ps2 = psV[m][:, 0:w].rearrange("p (h x) -> p h x", h=2)
                        m2 = m ^ 2
                        oVa = Va2[:, m, 0:2 * w].rearrange("p (h x) -> p h x", h=2)
                        nc.vector.tensor_copy(out=oVa[:, :, 0:r], in_=ps2)
                        oVa2 = Va2[:, m2, 0:2 * w].rearrange("p (h x) -> p h x", h=2)
                        nc.vector.tensor_copy(out=oVa2[:, :, r:2 * r], in_=ps2)
                        nc.scalar.mul(Vb2[:, m, 0:r], psV[m][:, r:w], -1.0)
                        nc.vector.tensor_copy(
                            out=Vb2[:, m, w:w + r], in_=psV[m][:, 0:r]
                        )
                        nc.scalar.mul(Vb2[:, m2, r:w], psV[m][:, r:w], -1.0)
                        nc.vector.tensor_copy(
                            out=Vb2[:, m2, w + r:2 * w], in_=psV[m][:, 0:r]
                        )
                    Ua, Ub, Va, Vb = Ua2, Ub2, Va2, Vb2
                else:
                    Uf = spool.tile([128, NK, 64], f32, tag="Ua")
                    for m in range(NK):
                        nc.vector.tensor_copy(out=Uf[:, m, :], in_=psU[m][:, 0:w])

            # --- measurement: G = V^dag V (as [-Gi | Gr]), W = U G, signed rowdots
            R = 32
            psG = pspool.tile([R, 2 * R], f32, tag="ps", name="psG")
            for k in range(NK):
                nc.tensor.matmul(
                    psG, lhsT=Va[:, k, 0:R], rhs=Vb[:, k, :], start=(k == 0),
                    stop=False,
                )
                nc.tensor.matmul(
                    psG, lhsT=Va[:, k, R:2 * R], rhs=Va[:, k, :], start=False,
                    stop=(k == NK - 1),
                )
            Gy = fpool.tile([R, 2 * R], f32, tag="Gy")
            nc.vector.tensor_copy(out=Gy[:], in_=psG[:])
            Gx = fpool.tile([R, 2 * R], f32, tag="Gx")
            nc.vector.tensor_copy(out=Gx[:, 0:R], in_=psG[:, R:2 * R])
            nc.scalar.mul(Gx[:, R:2 * R], psG[:, 0:R], -1.0)

            nsum = fpool.tile([128, 1], f32, tag="nsum")
            for m in range(NK):
                psT = pspool.tile([R, 256], f32, tag="ps", name=f"psT{m}")
                nc.tensor.transpose(psT[:, 0:128], Uf[:, m, 0:R], ident_t[:])
                nc.tensor.transpose(psT[:, 128:256], Uf[:, m, R:2 * R], ident_t[:])
                UT = fpool.tile([R, 256], f32, tag="UT")
                nc.vector.tensor_copy(out=UT[:], in_=psT[:])
                psW = pspool.tile([128, 2 * R], f32, tag="ps", name=f"psW{m}")
                nc.tensor.matmul(
                    psW, lhsT=UT[:, 0:128], rhs=Gx[:], start=True, stop=False
                )
                nc.tensor.matmul(
                    psW, lhsT=UT[:, 128:256], rhs=Gy[:], start=False, stop=True
                )
                t = fpool.tile([128, 2 * R], f32, tag="tdot")
                nc.vector.tensor_mul(out=t[:], in0=psW[:], in1=Uf[:, m, :])
                nm = fpool.tile([128, 1], f32, tag=f"nm{m}")
                nc.vector.reduce_sum(out=nm[:], in_=t[:], axis=X)
                if m == 0:
                    nc.vector.tensor_copy(out=nsum[:], in_=nm[:])
                elif m == 1:
                    nc.vector.tensor_add(out=nsum[:], in0=nsum[:], in1=nm[:])
                else:
                    nc.vector.tensor_sub(out=nsum[:], in0=nsum[:], in1=nm[:])

            psS = pspool.tile([1, 1], f32, tag="ps", name="psS")
            nc.tensor.matmul(
                psS, lhsT=nsum[:], rhs=masks_t[:, 4:5], start=True, stop=True
            )
            osb = fpool.tile([1, 1], f32, tag="osb")
            nc.vector.tensor_copy(out=osb[:], in_=psS[:])
            nc.sync.dma_start(out_d[:, :], osb[:])

    nc.finalize()
    return nc


def _get_bass():
    if "nc" not in _CACHE:
        _CACHE["nc"] = _build_bass()
    return _CACHE["nc"]


_IN_NAMES = ("wLr", "wLi", "wMr", "wMi", "ua0", "ub0", "va0", "vb0",
             "masks", "ident")


def kernel(inputs, params, basis):
    from concourse.bass_utils import run_bass_kernel_spmd

    d = build_host_data(params, basis)
    nc = _get_bass()
    in_map = {k: np.ascontiguousarray(d[k]) for k in _IN_NAMES}
    res = run_bass_kernel_spmd(
        nc, [dict(in_map) for _ in range(8)], core_ids=list(range(8))
    )
    _CACHE["last_results"] = res
    e = np.float32(res.results[0]["out"][0, 0])
    n = np.asarray(inputs).shape[0]
    return np.full((n,), e, dtype=np.float32)


# revision 9
# speedup vs baseline: 3.7897x; 3.7897x over previous
"""Trainium2 Bass kernel for nn_PennylaneCircuit: 18-qubit statevector circuit,
6 layers of per-wire RX/RY/RZ + nearest-neighbor CNOT chain, measuring <Z_0>.

Math: split wires 0-8 (rows) | 9-17 (cols); the state is a 512x512 complex
matrix S. Per layer, all row-space gates fuse into one 512x512 operator L_l
(host-precomputed from the angles), all col-space gates into M_l, and the only
cross-cut gate CNOT(8,9) makes the Schmidt rank double. So S = U @ V^T with
rank <= 32 factors; the device only ever multiplies 512x512 operators into
512x{2..32} factors and computes the final <Z_0> from the Gram matrix of V.

Device data layout: factors packed as f32 [real | imag] per 128-row tile;
complex matmul = two PSUM-accumulated passes (weights L_r, L_i transposed on
host) against packed rhs [Ur|Ui] and [-Ui|Ur].
"""

import numpy as np

NQ = 18
DEPTH = 6
A = 9            # row-group wires 0..8; col group 9..17
DIM = 1 << A     # 512
NK = DIM // 128  # 4 partition tiles per factor


# ---------------------------------------------------------------- host math

def _rx(t):
    c, s = np.cos(t / 2), np.sin(t / 2)
    return np.array([[c, -1j * s], [-1j * s, c]])


def _ry(t):
    c, s = np.cos(t / 2), np.sin(t / 2)
    return np.array([[c, -s], [s, c]])


def _rz(t):
    return np.array([[np.exp(-1j * t / 2), 0], [0, np.exp(1j * t / 2)]])


def _fused_1q(angles):
    # circuit order RX, RY, RZ  =>  matrix RZ @ RY @ RX
    return _rz(angles[2]) @ _ry(angles[1]) @ _rx(angles[0])


def _kron_all(mats):
    out = np.array([[1.0 + 0j]])
    for m in mats:
        out = np.kron(out, m)
    return out


def _cnot_chain_op(n, pairs):
    """Permutation matrix applying CNOT(w, w+1) for w in pairs, in order.

    Wire w of an n-wire group sits at bit (n-1-w) of the index."""
    dim = 1 << n
    perm = np.arange(dim)
    for w in pairs:
        cb, tb = n - 1 - w, n - 2 - w
        ctrl = (perm >> cb) & 1
        perm = np.where(ctrl == 1, perm ^ (1 << tb), perm)
    op = np.zeros((dim, dim), dtype=np.complex128)
    op[perm, np.arange(dim)] = 1
    return op


def _build_ops(params, basis):
    """L[0..5] row-space ops, M[0..4] col-space ops (M[5] never affects <Z_0>)."""
    params = np.asarray(params, np.float64)
    basis = np.asarray(basis, np.float64)
    Cleft = _cnot_chain_op(A, range(A - 1))
    Cright = _cnot_chain_op(NQ - A, range(NQ - A - 1))
    Ab = _kron_all([_fused_1q(basis[w]) for w in range(A)])
    Bb = _kron_all([_fused_1q(basis[A + w]) for w in range(NQ - A)])
    L, M = [], []
    for l in range(DEPTH):
        Rl = _kron_all([_fused_1q(params[l, w]) for w in range(A)])
        Rt = _kron_all([_fused_1q(params[l, A + w]) for w in range(NQ - A)])
        Al = Cleft @ Rl
        if l == 0:
            L.append(Al @ Ab)
            M.append(Rt @ Bb)
        else:
            L.append(Al)
            M.append(Rt @ Cright)
    return L, M


def build_host_data(params, basis):
    """Everything the device needs, all float32."""
    L, M = _build_ops(params, basis)
    u1 = L[0][:, 0]            # state after layer 1 = L1 e0 (x) M1 e0
    v1 = M[0][:, 0]
    # crossing X_1: U -> [mask_e*u1 | mask_o*u1],  V -> [v1 | P v1]
    idx = np.arange(DIM)
    me = (idx % 2 == 0).astype(np.float64)
    U2 = np.stack([u1 * me, u1 * (1 - me)], axis=1)
    V2 = np.stack([v1, v1[idx ^ 256]], axis=1)

    import ml_dtypes
    bf16 = ml_dtypes.bfloat16
    d = {
        "ua0": _pack_pair(U2.real, U2.imag).astype(bf16),
        "ub0": _pack_pair(-U2.imag, U2.real).astype(bf16),
        "va0": _pack_pair(V2.real, V2.imag).astype(bf16),
        "vb0": _pack_pair(-V2.imag, V2.real).astype(bf16),
    }

    d["wLr"] = np.stack([L[l].T.real for l in range(1, 6)]).astype(bf16)
    d["wLi"] = np.stack([L[l].T.imag for l in range(1, 6)]).astype(bf16)
    d["wMr"] = np.stack([M[l].T.real for l in range(1, 5)]).astype(bf16)
    d["wMi"] = np.stack([M[l].T.imag for l in range(1, 5)]).astype(bf16)

    masks = np.zeros((128, 8), np.float32)
    p = np.arange(128)
    masks[:, 0] = (p % 2 == 0)          # mE
    masks[:, 1] = (p % 2 == 1)          # mO
    masks[:, 2] = -masks[:, 0]          # -mE
    masks[:, 3] = -masks[:, 1]          # -mO
    masks[:, 4] = 1.0                   # ones
    d["masks"] = masks
    d["ident"] = np.eye(128).astype(bf16)
    return d


def _pack_pair(first, second):
    """two (512, r) real arrays -> (128, NK, 2r) f32 [first | second]."""
    r = first.shape[1]
    out = np.empty((128, NK, 2 * r), np.float32)
    for k in range(NK):
        out[:, k, :r] = first[k * 128:(k + 1) * 128]
        out[:, k, r:] = second[k * 128:(k + 1) * 128]
    return out


# ------------------------------------------------- numpy mirror of the device

def device_sim(d):
    """Mirrors the Bass op schedule exactly (packed tiles, two-pass cmms)."""
    d = {k: np.asarray(v, np.float32) for k, v in d.items()}
    wLr, wLi, wMr, wMi = d["wLr"], d["wLi"], d["wMr"], d["wMi"]
    masks = d["masks"]
    mE, mO = masks[:, 0:1], masks[:, 1:2]

    Ua, Ub = d["ua0"].copy(), d["ub0"].copy()
    Va, Vb = d["va0"].copy(), d["vb0"].copy()

    def cmm(wr, wi, Xa, Xb, r):
        """psum[m] = sum_k wr[k,m].T @ Xa[k] + wi[k,m].T @ Xb[k],  (128, 2r) each."""
        w = 2 * r
        ps = []
        for m in range(NK):
            acc = np.zeros((128, w), np.float32)
            for k in range(NK):
                lr = wr[k * 128:(k + 1) * 128, m * 128:(m + 1) * 128]
                li = wi[k * 128:(k + 1) * 128, m * 128:(m + 1) * 128]
                acc += lr.T @ Xa[:, k, :w] + li.T @ Xb[:, k, :w]
            ps.append(acc)
        return ps

    for li in range(5):
        r = 2 << li          # rank during this layer's cmm (2,4,8,16,32)
        w = 2 * r
        psU = cmm(wLr[li], wLi[li], Ua, Ub, r)
        if li < 4:
            psV = cmm(wMr[li], wMi[li], Va, Vb, r)
            Ua2 = np.zeros((128, NK, 4 * r), np.float32)
            Ub2 = np.zeros_like(Ua2)
            Va2 = np.zeros_like(Ua2)
            Vb2 = np.zeros_like(Ua2)
            for m in range(NK):
                pv = psU[m].reshape(128, 2, r)           # [Cr | Ci]
                oA = Ua2[:, m].reshape(128, 2, 2 * r)    # [Ur' | Ui']
                oA[:, :, 0:r] = pv * mE[:, None]
                oA[:, :, r:] = pv * mO[:, None]
                oB = Ub2[:, m].reshape(128, 2, 2 * r)    # [-Ui' | Ur']
                oB[:, 0, 0:r] = pv[:, 1] * -mE
                oB[:, 0, r:] = pv[:, 1] * -mO
                oB[:, 1, :] = oA[:, 0]
                # V: Va2[m] gets [Cr| ...] halves, Va2[m^2] the P-copy
                ps = psV[m].reshape(128, 2, r)
                Va2[:, m].reshape(128, 2, 2 * r)[:, :, 0:r] = ps
                Va2[:, m ^ 2].reshape(128, 2, 2 * r)[:, :, r:] = ps
                Vb2[:, m, 0:r] = -psV[m][:, r:]
                Vb2[:, m, 2 * r:3 * r] = psV[m][:, 0:r]
                Vb2[:, m ^ 2, r:2 * r] = -psV[m][:, r:]
                Vb2[:, m ^ 2, 3 * r:] = psV[m][:, 0:r]
            Ua, Ub, Va, Vb = Ua2, Ub2, Va2, Vb2
        else:
            Uf = np.zeros((128, NK, w), np.float32)
            for m in range(NK):
                Uf[:, m] = psU[m]

    # G accumulation: psG = [-Gi | Gr]  (32, 64)
    R = 32
    psG = np.zeros((R, 2 * R), np.float32)
    for k in range(NK):
        Vr, Vi = Va[:, k, :R], Va[:, k, R:]
        psG += Vr.T @ Vb[:, k] + Vi.T @ Va[:, k]
    Gy = psG                                  # [-Gi | Gr]
    Gx = np.concatenate([psG[:, R:], -psG[:, :R]], 1)   # [Gr | Gi]

    total = np.zeros((128, 1), np.float32)
    for m in range(NK):
        UrT = Uf[:, m, :R].T                  # (32, 128)
        UiT = Uf[:, m, R:].T
        psW = UrT.T @ Gx + UiT.T @ Gy         # [Wr | Wi] (128, 64)
        t = psW * Uf[:, m]                    # elementwise both halves
        nm = t.sum(axis=1, keepdims=True)
        total = total + nm if m < 2 else total - nm
    return np.float32(total.sum())


# ------------------------------------------------------------- bass program

_CACHE = {}


def _build_bass():
    import concourse.mybir as mybir
    import concourse.tile as tile
    from concourse import bacc

    f32 = mybir.dt.float32
    b16 = mybir.dt.bfloat16
    mult = mybir.AluOpType.mult
    X = mybir.AxisListType.X

    nc = bacc.Bacc("TRN2", target_bir_lowering=False)
    wLr_d = nc.dram_tensor("wLr", (5, 512, 512), b16, kind="ExternalInput")
    wLi_d = nc.dram_tensor("wLi", (5, 512, 512), b16, kind="ExternalInput")
    wMr_d = nc.dram_tensor("wMr", (4, 512, 512), b16, kind="ExternalInput")
    wMi_d = nc.dram_tensor("wMi", (4, 512, 512), b16, kind="ExternalInput")
    ua0_d = nc.dram_tensor("ua0", (128, NK, 4), b16, kind="ExternalInput")
    ub0_d = nc.dram_tensor("ub0", (128, NK, 4), b16, kind="ExternalInput")
    va0_d = nc.dram_tensor("va0", (128, NK, 4), b16, kind="ExternalInput")
    vb0_d = nc.dram_tensor("vb0", (128, NK, 4), b16, kind="ExternalInput")
    masks_d = nc.dram_tensor("masks", (128, 8), f32, kind="ExternalInput")
    ident_d = nc.dram_tensor("ident", (128, 128), b16, kind="ExternalInput")
    out_d = nc.dram_tensor("out", (1, 1), f32, kind="ExternalOutput")

    with tile.TileContext(nc) as tc:
        with (
            tc.tile_pool(name="consts", bufs=1) as cpool,
            tc.tile_pool(name="weights", bufs=2) as wpool,
            tc.tile_pool(name="state", bufs=2) as spool,
            tc.tile_pool(name="fin", bufs=4) as fpool,
            tc.tile_pool(name="psum", bufs=8, space="PSUM") as pspool,
        ):
            masks_t = cpool.tile([128, 8], f32)
            nc.sync.dma_start(masks_t[:], masks_d[:, :])
            ident_t = cpool.tile([128, 128], b16)
            nc.sync.dma_start(ident_t[:], ident_d[:, :])

            Ua = spool.tile([128, NK, 64], b16, tag="Ua")
            Ub = spool.tile([128, NK, 64], b16, tag="Ub")
            Va = spool.tile([128, NK, 64], b16, tag="Va")
            Vb = spool.tile([128, NK, 64], b16, tag="Vb")
            nc.sync.dma_start(Ua[:, :, 0:4], ua0_d[:, :, :])
            nc.sync.dma_start(Ub[:, :, 0:4], ub0_d[:, :, :])
            nc.sync.dma_start(Va[:, :, 0:4], va0_d[:, :, :])
            nc.sync.dma_start(Vb[:, :, 0:4], vb0_d[:, :, :])

            def load_w(dram, li, tag):
                t = wpool.tile([128, NK, 512], b16, tag=tag)
                nc.sync.dma_start(
                    t[:], dram[li, :, :].rearrange("(ko p) m -> p ko m", p=128)
                )
                return t

            def cmm(wr, wi, Xa, Xb, r, tag):
                w = 2 * r
                ps = []
                for m in range(NK):
                    pt = pspool.tile([128, 64], f32, tag="ps", name=f"{tag}{m}")
                    for k in range(NK):
                        nc.tensor.matmul(
                            pt[:, 0:w],
                            lhsT=wr[:, k, m * 128:(m + 1) * 128],
                            rhs=Xa[:, k, 0:w],
                            start=(k == 0),
                            stop=False,
                        )
                        nc.tensor.matmul(
                            pt[:, 0:w],
                            lhsT=wi[:, k, m * 128:(m + 1) * 128],
                            rhs=Xb[:, k, 0:w],
                            start=False,
                            stop=(k == NK - 1),
                        )
                    ps.append(pt)
                return ps

            mE = masks_t[:, 0:1, None]
            mO = masks_t[:, 1:2, None]
            mEn = masks_t[:, 2:3]
            mOn = masks_t[:, 3:4]

            Uf = None
            for li in range(5):
                r = 2 << li
                w = 2 * r
                wr = load_w(wLr_d, li, "wLr")
                wi = load_w(wLi_d, li, "wLi")
                psU = cmm(wr, wi, Ua, Ub, r, f"u{li}")
                if li < 4:
                    mr = load_w(wMr_d, li, "wMr")
                    mi = load_w(wMi_d, li, "wMi")
                    psV = cmm(mr, mi, Va, Vb, r, f"v{li}")
                    Ua2 = spool.tile([128, NK, 64], b16, tag="Ua")
                    Ub2 = spool.tile([128, NK, 64], b16, tag="Ub")
                    Va2 = spool.tile([128, NK, 64], b16, tag="Va")
                    Vb2 = spool.tile([128, NK, 64], b16, tag="Vb")
                    for m in range(NK):
                        pv = psU[m][:, 0:w].rearrange("p (h x) -> p h x", h=2)
                        oA = Ua2[:, m, 0:2 * w].rearrange("p (h x) -> p h x", h=2)
                        nc.vector.tensor_tensor(
                            out=oA[:, :, 0:r], in0=pv, in1=mE.to_broadcast((128, 2, r)),
                            op=mult,
                        )
                        nc.vector.tensor_tensor(
                            out=oA[:, :, r:2 * r], in0=pv,
                            in1=mO.to_broadcast((128, 2, r)), op=mult,
                        )
                        oB = Ub2[:, m, 0:2 * w].rearrange("p (h x) -> p h x", h=2)
                        nc.vector.tensor_tensor(
                            out=oB[:, 0, 0:r], in0=psU[m][:, r:w],
                            in1=mEn.to_broadcast((128, r)), op=mult,
                        )
                        nc.vector.tensor_tensor(
                            out=oB[:, 0, r:2 * r], in0=psU[m][:, r:w],
                            in1=mOn.to_broadcast((128, r)), op=mult,
                        )
                        nc.vector.tensor_copy(out=oB[:, 1, :], in_=oA[:, 0, :])
                        # V side: doubled copyback [V | PV]
                        ps2 = psV[m][:, 0:w].rearrange("p (h x) -> p h x", h=2)
                        m2 = m ^ 2
                        oVa = Va2[:, m, 0:2 * w].rearrange("p (h x) -> p h x", h=2)
                        nc.vector.tensor_copy(out=oVa[:, :, 0:r], in_=ps2)
                        oVa2 = Va2[:, m2, 0:2 * w].rearrange("p (h x) -> p h x", h=2)
                        nc.vector.tensor_copy(out=oVa2[:, :, r:2 * r], in_=ps2)
                        nc.scalar.mul(Vb2[:, m, 0:r], psV[m][:, r:w], -1.0)
                        nc.vector.tensor_copy(
                            out=Vb2[:, m, w:w + r], in_=psV[m][:, 0:r]
                        )
                        nc.scalar.mul(Vb2[:, m2, r:w], psV[m][:, r:w], -1.0)
                        nc.vector.tensor_copy(
                            out=Vb2[:, m2, w + r:2 * w], in_=psV[m][:, 0:r]
                        )
                    Ua, Ub, Va, Vb = Ua2, Ub2, Va2, Vb2
                else:
                    Uf = spool.tile([128, NK, 64], b16, tag="Ua")
                    for m in range(NK):
                        nc.vector.tensor_copy(out=Uf[:, m, :], in_=psU[m][:, 0:w])

            # --- measurement: G = V^dag V (as [-Gi | Gr]), W = U G, signed rowdots
            R = 32
            psG = pspool.tile([R, 2 * R], f32, tag="ps", name="psG")
            for k in range(NK):
                nc.tensor.matmul(
                    psG, lhsT=Va[:, k, 0:R], rhs=Vb[:, k, :], start=(k == 0),
                    stop=False,
                )
                nc.tensor.matmul(
                    psG, lhsT=Va[:, k, R:2 * R], rhs=Va[:, k, :], start=False,
                    stop=(k == NK - 1),
                )
            Gy = fpool.tile([R, 2 * R], b16, tag="Gy")
            nc.vector.tensor_copy(out=Gy[:], in_=psG[:])
            Gx = fpool.tile([R, 2 * R], b16, tag="Gx")
            nc.vector.tensor_copy(out=Gx[:, 0:R], in_=psG[:, R:2 * R])
            nc.scalar.mul(Gx[:, R:2 * R], psG[:, 0:R], -1.0)

            nsum = fpool.tile([128, 1], f32, tag="nsum")
            for m in range(NK):
                psT = pspool.tile([R, 256], b16, tag="ps", name=f"psT{m}")
                nc.tensor.transpose(psT[:, 0:128], Uf[:, m, 0:R], ident_t[:])
                nc.tensor.transpose(psT[:, 128:256], Uf[:, m, R:2 * R], ident_t[:])
                UT = fpool.tile([R, 256], b16, tag="UT")
                nc.vector.tensor_copy(out=UT[:], in_=psT[:])
                psW = pspool.tile([128, 2 * R], f32, tag="ps", name=f"psW{m}")
                nc.tensor.matmul(
                    psW, lhsT=UT[:, 0:128], rhs=Gx[:], start=True, stop=False
                )
                nc.tensor.matmul(
                    psW, lhsT=UT[:, 128:256], rhs=Gy[:], start=False, stop=True
                )
                t = fpool.tile([128, 2 * R], f32, tag="tdot")
                nc.vector.tensor_mul(out=t[:], in0=psW[:], in1=Uf[:, m, :])
                nm = fpool.tile([128, 1], f32, tag=f"nm{m}")
                nc.vector.reduce_sum(out=nm[:], in_=t[:], axis=X)
                if m == 0:
                    nc.vector.tensor_copy(out=nsum[:], in_=nm[:])
                elif m == 1:
                    nc.vector.tensor_add(out=nsum[:], in0=nsum[:], in1=nm[:])
                else:
                    nc.vector.tensor_sub(out=nsum[:], in0=nsum[:], in1=nm[:])

            psS = pspool.tile([1, 1], f32, tag="ps", name="psS")
            nc.tensor.matmul(
                psS, lhsT=nsum[:], rhs=masks_t[:, 4:5], start=True, stop=True
            )
            osb = fpool.tile([1, 1], f32, tag="osb")
            nc.vector.tensor_copy(out=osb[:], in_=psS[:])
            nc.sync.dma_start(out_d[:, :], osb[:])

    nc.finalize()
    return nc


def _get_bass():
    if "nc" not in _CACHE:
        _CACHE["nc"] = _build_bass()
    return _CACHE["nc"]


_IN_NAMES = ("wLr", "wLi", "wMr", "wMi", "ua0", "ub0", "va0", "vb0",
             "masks", "ident")


def kernel(inputs, params, basis):
    from concourse.bass_utils import run_bass_kernel_spmd

    d = build_host_data(params, basis)
    nc = _get_bass()
    in_map = {k: np.ascontiguousarray(d[k]) for k in _IN_NAMES}
    res = run_bass_kernel_spmd(
        nc, [dict(in_map) for _ in range(8)], core_ids=list(range(8))
    )
    _CACHE["last_results"] = res
    e = np.float32(res.results[0]["out"][0, 0])
    n = np.asarray(inputs).shape[0]
    return np.full((n,), e, dtype=np.float32)
